# revision 37
# baseline (speedup 1.0000x reference)
"""GQA attention (16 Q heads / 4 KV heads, RoPE, n=2048, d=64) on 8 trn2 cores.

Sharding: core c = (batch b=c//4, kv-group j=c%4). Each core owns 4 query
heads sharing one KV head, computes its partial output projection
(O_heads @ Wo_rows), and the host sums the 4 partials per batch.

On-device layout keeps head_dim on SBUF partitions (no activation
transposes):
  qt2 [128, 4*2048]  rows 0:64 = q^T (4 heads concat), rows 64:128 = copy
  kt2 [128, 2048]    rows 0:64 = k^T, rows 64:128 = copy
  S^T pair tiles [keys, 2x512 queries] via TWO CONCURRENT row-group
      matmuls (K=64 each, tile_position (0,0) / (64,0))
  P^T = exp(S^T/8): split between ACT (table exp) and DVE (Schraudolph
      bit-trick exp: bf16 bits = round((s*SCALE*log2e + 127 - sigma)*128))
  O^T+denom from matmul(lhsT=V_aug[keys,65], rhs=P^T)  (ones col -> denom)
Matmul inputs bf16, accumulation fp32 in PSUM.
"""

import os
import sys
import functools

import numpy as np

sys.path.insert(0, "/opt/trn_rl_repo")

import concourse.bass as bass  # noqa: E402
import concourse.bacc as bacc  # noqa: E402
import concourse.tile as tile  # noqa: E402
import concourse.mybir as mybir  # noqa: E402
from concourse.masks import make_identity  # noqa: E402

F32 = mybir.dt.float32
BF16 = mybir.dt.bfloat16
I16 = mybir.dt.int16
EXP = mybir.ActivationFunctionType.Exp
MULT = mybir.AluOpType.mult
ADD = mybir.AluOpType.add

B, N, DIM = 2, 2048, 1024
HEADS, KVH, D = 16, 4, 64
HPC = HEADS // KVH          # q heads per core = 4
SCALE = D ** -0.5           # 1/8
QTOT = HPC * N              # 8192 concatenated query columns
NKB = N // 128              # 16 key blocks
NPR = NKB // 2              # 8 key-block pairs
NDB = DIM // 128            # 8 contraction blocks for projections
NCH = QTOT // 512           # 16 query chunks of 512

# Schraudolph exp constants (bf16 bit domain): bits16 = round(s*A2 + B2)
SIGMA = 0.058621
A2 = SCALE * float(np.log2(np.e)) * 128.0
B2 = (127.0 - SIGMA) * 128.0
# which key-block pairs (of 8 per chunk) run exp on DVE instead of ACT
DVE_PAIRS = tuple(
    int(t) for t in os.environ.get("KERNEL_DVE_PAIRS", "2,4,6").split(",") if t
)
NO_PAIR = bool(os.environ.get("KERNEL_NO_PAIR"))
EXACT_RECIP = bool(os.environ.get("KERNEL_EXACT_RECIP"))
DUMP = bool(os.environ.get("KERNEL_DUMP"))

LAST_RESULTS = {}           # test.py introspection


def build_kernel(nc, tc, io):
    from contextlib import ExitStack

    xt, wq, wkv, wo = io["xt"], io["wq"], io["wkv"], io["wo"]
    cos2, sin2, out = io["cos2"], io["sin2"], io["out"]

    es = ExitStack()
    consts = es.enter_context(tc.tile_pool(name="consts", bufs=1))
    ot_pool = es.enter_context(tc.tile_pool(name="ot", bufs=1))
    qk_pool = es.enter_context(tc.tile_pool(name="qk", bufs=1))

    # --- constants / weights in SBUF ---
    wq_sb = consts.tile([128, NDB, 2 * 128], BF16, tag="wq")
    wkv_sb = consts.tile([128, NDB, 128], BF16, tag="wkv")
    wo_sb = consts.tile([128, 2, DIM], BF16, tag="wo")
    cos_sb = consts.tile([128, N], BF16, tag="cos")    # rows 64:128 = dup
    sin_sb = consts.tile([128, N], BF16, tag="sin")    # [-s,+s,-s,+s] x32
    id64 = consts.tile([64, 64], BF16, tag="id")
    nc.sync.dma_start(wkv_sb, wkv.transpose([1, 0, 2]))
    nc.sync.dma_start(cos_sb, cos2)
    nc.sync.dma_start(sin_sb, sin2)
    make_identity(nc, id64)

    # --- activations ---
    qt_sb = qk_pool.tile([128, QTOT], BF16, tag="qt")
    kt_sb = qk_pool.tile([128, N], BF16, tag="kt")
    vaug_sb = qk_pool.tile([128, NKB, 128], BF16, tag="vaug")
    ot_sb = [
        ot_pool.tile([128, N], BF16, tag=f"ot{i}", name=f"ot{i}") for i in range(2)
    ]
    # vT staging aliases into ot_sb[0] (free until attention writes it)
    vt_sb = ot_sb[0][0:64, :]

    with (
        tc.tile_pool(name="xt", bufs=1) as xt_pool,
        tc.tile_pool(name="ropetmp", bufs=2) as rope_tmp,
        tc.tile_pool(name="proj", bufs=3, space="PSUM") as ppq,
    ):
        xt_sb = xt_pool.tile([128, NDB, N], BF16, tag="xt")
        # ch-major so KV proj for ch 0 can start after 1/4 of the loads;
        # wq/wo only needed later, so they load after the first x chunk
        for ch in range(4):
            for kb in range(NDB):
                nc.sync.dma_start(
                    xt_sb[:, kb, ch * 512:(ch + 1) * 512],
                    xt[kb, :, ch * 512:(ch + 1) * 512],
                )
            if ch == 0:
                nc.sync.dma_start(wq_sb, wq.transpose([1, 0, 2]))
            elif ch == 1:
                nc.sync.dma_start(wo_sb, wo.transpose([1, 0, 2]))

        def rope64(dst, src, ch):
            """dst[64,512] SBUF bf16 <- RoPE(src[64,512] PSUM f32).

            bf16 staging (on ACT, which is idle pre-attention) first: the
            tensor-tensor work then runs in DVE 2x_1P packed mode."""
            cs = cos_sb[0:64, ch * 512:(ch + 1) * 512]
            sn = sin_sb[0:64, ch * 512:(ch + 1) * 512]
            sb = rope_tmp.tile([64, 512], BF16, tag="t0")
            t1 = rope_tmp.tile([64, 512], BF16, tag="t1")
            t2 = rope_tmp.tile([64, 512], BF16, tag="t2")
            nc.scalar.copy(sb, src)
            nc.vector.tensor_mul(t1, sb, cs)
            # sin table is pre-swapped so both SBUF inputs share a base
            # partition (walrus NCC_IBIR297 requirement)
            nc.vector.tensor_mul(t2[0:32, :], sb[32:64, :], sn[32:64, :])
            nc.vector.tensor_mul(t2[32:64, :], sb[0:32, :], sn[0:32, :])
            nc.gpsimd.tensor_add(dst, t1, t2)

        # KV projection (k rows 0:64, v rows 64:128 of the pack).
        for ch in range(4):
            pkv = ppq.tile([128, 512], F32, tag="pj")
            for kb in range(NDB):
                nc.tensor.matmul(
                    pkv,
                    wkv_sb[:, kb, :],
                    xt_sb[:, kb, ch * 512:(ch + 1) * 512],
                    start=(kb == 0),
                    stop=(kb == NDB - 1),
                )
            rope64(kt_sb[0:64, ch * 512:(ch + 1) * 512], pkv[0:64, :], ch)
            nc.scalar.copy(
                vt_sb[:, ch * 512:(ch + 1) * 512], pkv[64:128, :]
            )
        # duplicate k^T onto partitions 64:128 (row-paired S matmuls)
        nc.sync.dma_start(kt_sb[64:128, :], kt_sb[0:64, :])

        # V_aug: transpose vT -> [keys,64] blocks (ones cols below).
        for t in range(NKB):
            ptr = ppq.tile([128, 64], BF16, tag="pjt")
            nc.tensor.transpose(
                ptr[:, 0:64], vt_sb[:, t * 128:(t + 1) * 128], id64
            )
            nc.vector.tensor_copy(vaug_sb[:, t, 64:128], ptr[:, 0:64])
        # cols 0:64 all ones -> PV matmul replicates the softmax
        # denominator onto output partitions 0:64 (base_partition 0:
        # custom-DVE recip mishandles nonzero base partitions on HW)
        nc.gpsimd.memset(vaug_sb[:, :, 0:64], 1.0)

        def qproj_ch(pack, ch):
            """Q proj + stacked 2-head rope for one (head-pair, n-chunk)."""
            pq = ppq.tile([128, 512], F32, tag="pj")
            for kb in range(NDB):
                nc.tensor.matmul(
                    pq,
                    wq_sb[:, kb, pack * 128:(pack + 1) * 128],
                    xt_sb[:, kb, ch * 512:(ch + 1) * 512],
                    start=(kb == 0),
                    stop=(kb == NDB - 1),
                )
            cs = cos_sb[:, ch * 512:(ch + 1) * 512]
            sn = sin_sb[:, ch * 512:(ch + 1) * 512]
            qb = rope_tmp.tile([128, 512], BF16, tag="qt0")
            t1 = rope_tmp.tile([128, 512], BF16, tag="qt1")
            t2 = rope_tmp.tile([128, 512], BF16, tag="qt2")
            nc.scalar.copy(qb, pq)
            nc.vector.tensor_mul(t1, qb, cs)
            nc.vector.tensor_mul(t2[0:32, :], qb[32:64, :], sn[32:64, :])
            nc.vector.tensor_mul(t2[32:64, :], qb[0:32, :], sn[0:32, :])
            nc.vector.tensor_mul(t2[64:96, :], qb[96:128, :], sn[96:128, :])
            nc.vector.tensor_mul(t2[96:128, :], qb[64:96, :], sn[64:96, :])
            for hh in range(2):
                h = pack * 2 + hh
                dst = qt_sb[0:64, h * N + ch * 512: h * N + (ch + 1) * 512]
                # head-even add on (idle) gpsimd, base partition 0 only
                eng = nc.gpsimd if hh == 0 else nc.vector
                eng.tensor_add(
                    dst, t1[hh * 64:(hh + 1) * 64, :],
                    t2[hh * 64:(hh + 1) * 64, :],
                )
                nc.sync.dma_start(
                    qt_sb[64:128, h * N + ch * 512: h * N + (ch + 1) * 512],
                    dst,
                )

        for pack in range(2):
            for ch in range(4):
                qproj_ch(pack, ch)

    # --- attention ---
    # Software-pipelined pair stream: S matmuls are emitted two pairs
    # ahead so the strict-FIFO PE queue doesn't stall on a PV waiting
    # for its exp.
    with (
        tc.tile_pool(name="small", bufs=3) as small,
        tc.tile_pool(name="ppool", bufs=6) as ppool,
        tc.tile_pool(name="psS", bufs=3, space="PSUM") as psS,
        tc.tile_pool(name="psO", bufs=2, space="PSUM") as psO,
    ):
        pairs = [(qc, pr) for qc in range(NCH) for pr in range(NPR)]
        ps_tiles = {}
        po_tiles = {}

        def emit_S(i):
            qc, pr = pairs[i]
            c0 = qc * 512
            ps_t = psS.tile([128, 1024], F32, tag="s", name=f"ps{i}")
            ps_tiles[i] = ps_t
            nc.tensor.matmul(
                ps_t[:, 0:512],
                kt_sb[0:64, pr * 256: pr * 256 + 128],
                qt_sb[0:64, c0:c0 + 512],
                start=True, stop=True,
            )
            if NO_PAIR:
                nc.tensor.matmul(
                    ps_t[:, 512:1024],
                    kt_sb[0:64, pr * 256 + 128: pr * 256 + 256],
                    qt_sb[0:64, c0:c0 + 512],
                    start=True, stop=True,
                )
            else:
                nc.tensor.matmul(
                    ps_t[:, 512:1024],
                    kt_sb[64:128, pr * 256 + 128: pr * 256 + 256],
                    qt_sb[64:128, c0:c0 + 512],
                    start=True, stop=True,
                )

        LOOKAHEAD = 2
        for i in range(LOOKAHEAD):
            emit_S(i)
        for i, (qc, pr) in enumerate(pairs):
            if i + LOOKAHEAD < len(pairs):
                emit_S(i + LOOKAHEAD)
            ps_t = ps_tiles.pop(i)
            if qc not in po_tiles:
                po_tiles[qc] = psO.tile([128, 512], F32, tag="o", name=f"po{qc}")
            po_t = po_tiles[qc]
            p_t = ppool.tile([128, 1024], BF16, tag="p")
            # alternate exp engines pair-by-pair so adjacent exps overlap;
            # pair 7 stays on ACT so the chunk-end normalize (DVE) slots
            # into the DVE idle window instead of delaying the next chunk
            dve_pairs = (1, 3, 5) if qc % 2 == 0 else (1, 3, 5, 6)
            if pr in dve_pairs:
                nc.vector.tensor_scalar(
                    p_t.bitcast(I16), ps_t, A2, B2, op0=MULT, op1=ADD
                )
            else:
                nc.scalar.activation(p_t, ps_t, EXP, bias=0.0, scale=SCALE)
            for half in range(2):
                nc.tensor.matmul(
                    po_t,
                    vaug_sb[:, 2 * pr + half, :],
                    p_t[:, half * 512:(half + 1) * 512],
                    start=(pr == 0 and half == 0),
                    stop=(pr == NPR - 1 and half == 1),
                    skip_group_check=True,
                )
            if pr == NPR - 1:
                # normalize: O^T / denom (denom on psum partitions 0:64)
                po_tiles.pop(qc)
                h = qc // 4
                pair, row0 = h // 2, 64 * (h % 2)
                col0 = (qc % 4) * 512
                rc = small.tile([64, 512], F32, tag="rc")
                if EXACT_RECIP:
                    nc.vector.reciprocal(rc, po_t[0:64, :])
                else:
                    nc.vector.reciprocal_approx_fast(rc, po_t[0:64, :])
                nc.vector.tensor_mul(
                    ot_sb[pair][row0:row0 + 64, col0:col0 + 512],
                    po_t[64:128, :],
                    rc,
                )

    # --- output projection: out[q, :] = sum_pair O^T_pair.T @ Wo_pair ---
    with (
        tc.tile_pool(name="pout", bufs=4, space="PSUM") as pout,
        tc.tile_pool(name="ostage", bufs=4) as ostage,
    ):
        for qb in range(N // 128):
            for nch in range(2):
                pt = pout.tile([128, 512], F32, tag="po")
                for pair in range(2):
                    nc.tensor.matmul(
                        pt,
                        ot_sb[pair][:, qb * 128:(qb + 1) * 128],
                        wo_sb[:, pair, nch * 512:(nch + 1) * 512],
                        start=(pair == 0),
                        stop=(pair == 1),
                    )
                st = ostage.tile([128, 512], F32, tag="st")
                if nch == 0:
                    nc.vector.tensor_copy(st, pt)
                else:
                    nc.scalar.copy(st, pt)
                nc.sync.dma_start(
                    out[qb * 128:(qb + 1) * 128, nch * 512:(nch + 1) * 512], st
                )

    if DUMP:
        nc.sync.dma_start(io["dqt"], qt_sb)
        nc.sync.dma_start(io["dkt"], kt_sb)
        nc.sync.dma_start(io["dvaug"], vaug_sb.rearrange("p a b -> p (a b)"))
        nc.sync.dma_start(io["dot0"], ot_sb[0])
        nc.sync.dma_start(io["dot1"], ot_sb[1])

    es.close()


def _rope_tables():
    inv_freq = 1.0 / (10000.0 ** (np.arange(0, D, 2, dtype=np.float64) / D))
    freqs = np.outer(np.arange(N, dtype=np.float64), inv_freq)  # [N, 32]
    cos_h = np.cos(freqs).astype(np.float32).T                  # [32, N]
    sin_h = np.sin(freqs).astype(np.float32).T                  # [32, N]
    cos64 = np.concatenate([cos_h, cos_h], 0)                   # [64, N]
    # swapped sin layout: row p holds the sin factor for the mul whose
    # *source* q rows sit at partition p (so both SBUF inputs share a base)
    sin64 = np.concatenate([sin_h, -sin_h], 0)                  # [64, N]
    cos2 = np.concatenate([cos64, cos64], 0)                    # [128, N]
    sin2 = np.concatenate([sin64, sin64], 0)                    # [128, N]
    return np.ascontiguousarray(cos2), np.ascontiguousarray(sin2)


@functools.lru_cache(maxsize=1)
def _program():
    nc = bacc.Bacc(
        "TRN2", target_bir_lowering=False, debug=False, enable_asserts=False
    )
    io = {
        "xt": nc.dram_tensor("xt", [NDB, 128, N], BF16, kind="ExternalInput").ap(),
        "wq": nc.dram_tensor("wq", [NDB, 128, 256], BF16, kind="ExternalInput").ap(),
        "wkv": nc.dram_tensor("wkv", [NDB, 128, 128], BF16, kind="ExternalInput").ap(),
        "wo": nc.dram_tensor("wo", [2, 128, DIM], BF16, kind="ExternalInput").ap(),
        "cos2": nc.dram_tensor("cos2", [128, N], BF16, kind="ExternalInput").ap(),
        "sin2": nc.dram_tensor("sin2", [128, N], BF16, kind="ExternalInput").ap(),
        "out": nc.dram_tensor("out", [N, DIM], F32, kind="ExternalOutput").ap(),
    }
    if DUMP:
        io["dqt"] = nc.dram_tensor("dqt", [128, QTOT], BF16, kind="ExternalOutput").ap()
        io["dkt"] = nc.dram_tensor("dkt", [128, N], BF16, kind="ExternalOutput").ap()
        io["dvaug"] = nc.dram_tensor("dvaug", [128, NKB * 128], BF16, kind="ExternalOutput").ap()
        io["dot0"] = nc.dram_tensor("dot0", [128, N], BF16, kind="ExternalOutput").ap()
        io["dot1"] = nc.dram_tensor("dot1", [128, N], BF16, kind="ExternalOutput").ap()
    with tile.TileContext(nc) as tc:
        build_kernel(nc, tc, io)
    nc.compile()
    return nc


def make_in_maps(x, Wq, Wkv, Wo):
    import ml_dtypes

    bf16 = ml_dtypes.bfloat16
    cos2, sin2 = _rope_tables()
    in_maps = []
    for c in range(8):
        b, j = c // 4, c % 4
        xt = np.ascontiguousarray(x[b].T).reshape(NDB, 128, N)
        wq_c = np.ascontiguousarray(Wq[:, 256 * j:256 * (j + 1)]).reshape(
            NDB, 128, 256
        )
        wkv_c = np.ascontiguousarray(
            np.concatenate(
                [Wkv[:, 64 * j:64 * (j + 1)],
                 Wkv[:, 256 + 64 * j:256 + 64 * (j + 1)]],
                axis=1,
            )
        ).reshape(NDB, 128, 128)
        wo_c = np.ascontiguousarray(Wo[256 * j:256 * (j + 1), :]).reshape(
            2, 128, DIM
        )
        in_maps.append(
            {
                "xt": xt.astype(bf16),
                "wq": wq_c.astype(bf16),
                "wkv": wkv_c.astype(bf16),
                "wo": wo_c.astype(bf16),
                "cos2": cos2.astype(bf16),
                "sin2": sin2.astype(bf16),
            }
        )
    return in_maps


def _install_ntff_hook():
    """Register the axon NTFF profiling hook that this image's antenv lacks."""
    import types

    if "antenv.axon_hooks" in sys.modules:
        return
    try:
        sys.path.append("/root/.axon_site")
        from trn_agent_boot.trn_boot import _ntff_profile_via_ctypes

        hook = _ntff_profile_via_ctypes("/opt/axon/libaxon_pjrt.so")
    except Exception:
        hook = None
    finally:
        try:
            sys.path.remove("/root/.axon_site")
        except ValueError:
            pass
    mod = types.ModuleType("antenv.axon_hooks")
    mod.get_axon_ntff_profile_hook = lambda: hook
    mod.set_axon_ntff_profile_hook = lambda h: None
    sys.modules["antenv.axon_hooks"] = mod
    # artifact upload needs bucket credentials this container lacks
    import concourse.bass_utils as bu

    bu.upload_artifacts = lambda tmpdir: "local://" + str(tmpdir)


def kernel(x, Wq, Wkv, Wo, bo):
    from concourse.bass_utils import run_bass_kernel_spmd

    _install_ntff_hook()
    nc = _program()
    in_maps = make_in_maps(x, Wq, Wkv, Wo)
    trace = bool(os.environ.get("KERNEL_TRACE"))
    res = run_bass_kernel_spmd(
        nc, in_maps, list(range(8)), trace=trace
    )
    LAST_RESULTS["res"] = res
    full = np.zeros((B, N, DIM), np.float32)
    for c in range(8):
        full[c // 4] += res.results[c]["out"]
    full += bo.astype(np.float32)
    return full


# revision 38
# speedup vs baseline: 1.0386x; 1.0386x over previous
"""GQA attention (16 Q heads / 4 KV heads, RoPE, n=2048, d=64) on 8 trn2 cores.

Sharding: core c = (batch b=c//4, kv-group j=c%4). Each core owns 4 query
heads sharing one KV head, computes its partial output projection
(O_heads @ Wo_rows), and the host sums the 4 partials per batch.

On-device layout keeps head_dim on SBUF partitions (no activation
transposes):
  qt2 [128, 4*2048]  rows 0:64 = q^T (4 heads concat), rows 64:128 = copy
  kt2 [128, 2048]    rows 0:64 = k^T, rows 64:128 = copy
  S^T pair tiles [keys, 2x512 queries] via TWO CONCURRENT row-group
      matmuls (K=64 each, tile_position (0,0) / (64,0))
  P^T = exp(S^T/8): split between ACT (table exp) and DVE (Schraudolph
      bit-trick exp: bf16 bits = round((s*SCALE*log2e + 127 - sigma)*128))
  O^T+denom from matmul(lhsT=V_aug[keys,65], rhs=P^T)  (ones col -> denom)
Matmul inputs bf16, accumulation fp32 in PSUM.
"""

import os
import sys
import functools

import numpy as np

sys.path.insert(0, "/opt/trn_rl_repo")

import concourse.bass as bass  # noqa: E402
import concourse.bacc as bacc  # noqa: E402
import concourse.tile as tile  # noqa: E402
import concourse.mybir as mybir  # noqa: E402
from concourse.masks import make_identity  # noqa: E402

F32 = mybir.dt.float32
BF16 = mybir.dt.bfloat16
I16 = mybir.dt.int16
EXP = mybir.ActivationFunctionType.Exp
MULT = mybir.AluOpType.mult
ADD = mybir.AluOpType.add

B, N, DIM = 2, 2048, 1024
HEADS, KVH, D = 16, 4, 64
HPC = HEADS // KVH          # q heads per core = 4
SCALE = D ** -0.5           # 1/8
QTOT = HPC * N              # 8192 concatenated query columns
NKB = N // 128              # 16 key blocks
NPR = NKB // 2              # 8 key-block pairs
NDB = DIM // 128            # 8 contraction blocks for projections
NCH = QTOT // 512           # 16 query chunks of 512

# Schraudolph exp constants (bf16 bit domain): bits16 = round(s*A2 + B2)
SIGMA = 0.058621
A2 = SCALE * float(np.log2(np.e)) * 128.0
B2 = (127.0 - SIGMA) * 128.0
# which key-block pairs (of 8 per chunk) run exp on DVE instead of ACT
DVE_PAIRS = tuple(
    int(t) for t in os.environ.get("KERNEL_DVE_PAIRS", "2,4,6").split(",") if t
)
NO_PAIR = bool(os.environ.get("KERNEL_NO_PAIR"))
EXACT_RECIP = bool(os.environ.get("KERNEL_EXACT_RECIP"))
DUMP = bool(os.environ.get("KERNEL_DUMP"))

LAST_RESULTS = {}           # test.py introspection


def build_kernel(nc, tc, io):
    from contextlib import ExitStack

    xt, wq, wkv, wo = io["xt"], io["wq"], io["wkv"], io["wo"]
    cos2, sin2, out = io["cos2"], io["sin2"], io["out"]

    es = ExitStack()
    consts = es.enter_context(tc.tile_pool(name="consts", bufs=1))
    ot_pool = es.enter_context(tc.tile_pool(name="ot", bufs=1))
    qk_pool = es.enter_context(tc.tile_pool(name="qk", bufs=1))

    # --- constants / weights in SBUF ---
    wq_sb = consts.tile([128, NDB, 2 * 128], BF16, tag="wq")
    wkv_sb = consts.tile([128, NDB, 128], BF16, tag="wkv")
    wo_sb = consts.tile([128, 2, DIM], BF16, tag="wo")
    cos_sb = consts.tile([128, N], BF16, tag="cos")    # rows 64:128 = dup
    sin_sb = consts.tile([128, N], BF16, tag="sin")    # [-s,+s,-s,+s] x32
    id64 = consts.tile([64, 64], BF16, tag="id")
    nc.sync.dma_start(wkv_sb, wkv.transpose([1, 0, 2]))
    nc.sync.dma_start(cos_sb, cos2)
    nc.sync.dma_start(sin_sb, sin2)
    make_identity(nc, id64)

    # --- activations ---
    qt_sb = qk_pool.tile([128, QTOT], BF16, tag="qt")
    kt_sb = qk_pool.tile([128, N], BF16, tag="kt")
    vaug_sb = qk_pool.tile([128, NKB, 128], BF16, tag="vaug")
    ot_sb = [
        ot_pool.tile([128, N], BF16, tag=f"ot{i}", name=f"ot{i}") for i in range(2)
    ]
    # vT staging aliases into ot_sb[0] (free until attention writes it)
    vt_sb = ot_sb[0][0:64, :]

    with (
        tc.tile_pool(name="xt", bufs=1) as xt_pool,
        tc.tile_pool(name="ropetmp", bufs=2) as rope_tmp,
        tc.tile_pool(name="proj", bufs=3, space="PSUM") as ppq,
    ):
        xt_sb = xt_pool.tile([128, NDB, N], BF16, tag="xt")
        # ch-major so KV proj for ch 0 can start after 1/4 of the loads;
        # wq/wo only needed later, so they load after the first x chunk
        for ch in range(4):
            for kb in range(NDB):
                nc.sync.dma_start(
                    xt_sb[:, kb, ch * 512:(ch + 1) * 512],
                    xt[kb, :, ch * 512:(ch + 1) * 512],
                )
            if ch == 0:
                nc.sync.dma_start(wq_sb, wq.transpose([1, 0, 2]))
            elif ch == 1:
                nc.sync.dma_start(wo_sb, wo.transpose([1, 0, 2]))

        def rope64(dst, src, ch):
            """dst[64,512] SBUF bf16 <- RoPE(src[64,512] PSUM f32).

            bf16 staging (on ACT, which is idle pre-attention) first: the
            tensor-tensor work then runs in DVE 2x_1P packed mode."""
            cs = cos_sb[0:64, ch * 512:(ch + 1) * 512]
            sn = sin_sb[0:64, ch * 512:(ch + 1) * 512]
            sb = rope_tmp.tile([64, 512], BF16, tag="t0")
            t1 = rope_tmp.tile([64, 512], BF16, tag="t1")
            t2 = rope_tmp.tile([64, 512], BF16, tag="t2")
            nc.scalar.copy(sb, src)
            nc.gpsimd.tensor_mul(t1, sb, cs)
            # sin table is pre-swapped so both SBUF inputs share a base
            # partition (walrus NCC_IBIR297 requirement)
            nc.vector.tensor_mul(t2[0:32, :], sb[32:64, :], sn[32:64, :])
            nc.vector.tensor_mul(t2[32:64, :], sb[0:32, :], sn[0:32, :])
            nc.gpsimd.tensor_add(dst, t1, t2)

        def kv_ch(ch):
            """KV proj + K rope + vT stage + V transposes for one n-chunk."""
            pkv = ppq.tile([128, 512], F32, tag="pj")
            for kb in range(NDB):
                nc.tensor.matmul(
                    pkv,
                    wkv_sb[:, kb, :],
                    xt_sb[:, kb, ch * 512:(ch + 1) * 512],
                    start=(kb == 0),
                    stop=(kb == NDB - 1),
                )
            rope64(kt_sb[0:64, ch * 512:(ch + 1) * 512], pkv[0:64, :], ch)
            nc.scalar.copy(
                vt_sb[:, ch * 512:(ch + 1) * 512], pkv[64:128, :]
            )
            for t in range(4 * ch, 4 * ch + 4):
                ptr = ppq.tile([128, 64], BF16, tag="pjt")
                nc.tensor.transpose(
                    ptr[:, 0:64], vt_sb[:, t * 128:(t + 1) * 128], id64
                )
                nc.vector.tensor_copy(vaug_sb[:, t, 64:128], ptr[:, 0:64])

        def qproj_ch(pack, ch):
            """Q proj + stacked 2-head rope for one (head-pair, n-chunk)."""
            pq = ppq.tile([128, 512], F32, tag="pj")
            for kb in range(NDB):
                nc.tensor.matmul(
                    pq,
                    wq_sb[:, kb, pack * 128:(pack + 1) * 128],
                    xt_sb[:, kb, ch * 512:(ch + 1) * 512],
                    start=(kb == 0),
                    stop=(kb == NDB - 1),
                )
            cs = cos_sb[:, ch * 512:(ch + 1) * 512]
            sn = sin_sb[:, ch * 512:(ch + 1) * 512]
            qb = rope_tmp.tile([128, 512], BF16, tag="qt0")
            t1 = rope_tmp.tile([128, 512], BF16, tag="qt1")
            t2 = rope_tmp.tile([128, 512], BF16, tag="qt2")
            nc.scalar.copy(qb, pq)
            nc.gpsimd.tensor_mul(t1, qb, cs)
            nc.vector.tensor_mul(t2[0:32, :], qb[32:64, :], sn[32:64, :])
            nc.vector.tensor_mul(t2[32:64, :], qb[0:32, :], sn[0:32, :])
            nc.vector.tensor_mul(t2[64:96, :], qb[96:128, :], sn[96:128, :])
            nc.vector.tensor_mul(t2[96:128, :], qb[64:96, :], sn[64:96, :])
            for hh in range(2):
                h = pack * 2 + hh
                dst = qt_sb[0:64, h * N + ch * 512: h * N + (ch + 1) * 512]
                # head-even add on (idle) gpsimd, base partition 0 only
                eng = nc.gpsimd if hh == 0 else nc.vector
                eng.tensor_add(
                    dst, t1[hh * 64:(hh + 1) * 64, :],
                    t2[hh * 64:(hh + 1) * 64, :],
                )
                nc.sync.dma_start(
                    qt_sb[64:128, h * N + ch * 512: h * N + (ch + 1) * 512],
                    dst,
                )

        for ch in range(4):
            kv_ch(ch)
            qproj_ch(0, ch)
        # duplicate k^T onto partitions 64:128 (row-paired S matmuls)
        nc.sync.dma_start(kt_sb[64:128, :], kt_sb[0:64, :])
        # cols 0:64 all ones -> PV matmul replicates the softmax
        # denominator onto output partitions 0:64 (base_partition 0:
        # custom-DVE recip mishandles nonzero base partitions on HW)
        nc.gpsimd.memset(vaug_sb[:, :, 0:64], 1.0)
        for ch in range(4):
            qproj_ch(1, ch)

    # --- attention ---
    # Software-pipelined pair stream: S matmuls are emitted two pairs
    # ahead so the strict-FIFO PE queue doesn't stall on a PV waiting
    # for its exp.
    with (
        tc.tile_pool(name="small", bufs=3) as small,
        tc.tile_pool(name="ppool", bufs=6) as ppool,
        tc.tile_pool(name="psS", bufs=3, space="PSUM") as psS,
        tc.tile_pool(name="psO", bufs=2, space="PSUM") as psO,
    ):
        pairs = [(qc, pr) for qc in range(NCH) for pr in range(NPR)]
        ps_tiles = {}
        po_tiles = {}

        def emit_S(i):
            qc, pr = pairs[i]
            c0 = qc * 512
            ps_t = psS.tile([128, 1024], F32, tag="s", name=f"ps{i}")
            ps_tiles[i] = ps_t
            nc.tensor.matmul(
                ps_t[:, 0:512],
                kt_sb[0:64, pr * 256: pr * 256 + 128],
                qt_sb[0:64, c0:c0 + 512],
                start=True, stop=True,
            )
            if NO_PAIR:
                nc.tensor.matmul(
                    ps_t[:, 512:1024],
                    kt_sb[0:64, pr * 256 + 128: pr * 256 + 256],
                    qt_sb[0:64, c0:c0 + 512],
                    start=True, stop=True,
                )
            else:
                nc.tensor.matmul(
                    ps_t[:, 512:1024],
                    kt_sb[64:128, pr * 256 + 128: pr * 256 + 256],
                    qt_sb[64:128, c0:c0 + 512],
                    start=True, stop=True,
                )

        LOOKAHEAD = 2
        for i in range(LOOKAHEAD):
            emit_S(i)
        for i, (qc, pr) in enumerate(pairs):
            if i + LOOKAHEAD < len(pairs):
                emit_S(i + LOOKAHEAD)
            ps_t = ps_tiles.pop(i)
            if qc not in po_tiles:
                po_tiles[qc] = psO.tile([128, 512], F32, tag="o", name=f"po{qc}")
            po_t = po_tiles[qc]
            p_t = ppool.tile([128, 1024], BF16, tag="p")
            # alternate exp engines pair-by-pair so adjacent exps overlap
            dve_pairs = (1, 3, 5) if qc % 2 == 0 else (1, 3, 5, 7)
            if pr in dve_pairs:
                nc.vector.tensor_scalar(
                    p_t.bitcast(I16), ps_t, A2, B2, op0=MULT, op1=ADD
                )
            else:
                nc.scalar.activation(p_t, ps_t, EXP, bias=0.0, scale=SCALE)
            for half in range(2):
                nc.tensor.matmul(
                    po_t,
                    vaug_sb[:, 2 * pr + half, :],
                    p_t[:, half * 512:(half + 1) * 512],
                    start=(pr == 0 and half == 0),
                    stop=(pr == NPR - 1 and half == 1),
                    skip_group_check=True,
                )
            if pr == NPR - 1:
                # normalize: O^T / denom (denom on psum partitions 0:64)
                po_tiles.pop(qc)
                h = qc // 4
                pair, row0 = h // 2, 64 * (h % 2)
                col0 = (qc % 4) * 512
                rc = small.tile([64, 512], F32, tag="rc")
                if EXACT_RECIP:
                    nc.vector.reciprocal(rc, po_t[0:64, :])
                else:
                    nc.vector.reciprocal_approx_fast(rc, po_t[0:64, :])
                nc.vector.tensor_mul(
                    ot_sb[pair][row0:row0 + 64, col0:col0 + 512],
                    po_t[64:128, :],
                    rc,
                )

    # --- output projection: out[q, :] = sum_pair O^T_pair.T @ Wo_pair ---
    with (
        tc.tile_pool(name="pout", bufs=4, space="PSUM") as pout,
        tc.tile_pool(name="ostage", bufs=4) as ostage,
    ):
        for qb in range(N // 128):
            for nch in range(2):
                pt = pout.tile([128, 512], F32, tag="po")
                for pair in range(2):
                    nc.tensor.matmul(
                        pt,
                        ot_sb[pair][:, qb * 128:(qb + 1) * 128],
                        wo_sb[:, pair, nch * 512:(nch + 1) * 512],
                        start=(pair == 0),
                        stop=(pair == 1),
                    )
                st = ostage.tile([128, 512], F32, tag="st")
                if nch == 0:
                    nc.vector.tensor_copy(st, pt)
                else:
                    nc.scalar.copy(st, pt)
                nc.sync.dma_start(
                    out[qb * 128:(qb + 1) * 128, nch * 512:(nch + 1) * 512], st
                )

    if DUMP:
        nc.sync.dma_start(io["dqt"], qt_sb)
        nc.sync.dma_start(io["dkt"], kt_sb)
        nc.sync.dma_start(io["dvaug"], vaug_sb.rearrange("p a b -> p (a b)"))
        nc.sync.dma_start(io["dot0"], ot_sb[0])
        nc.sync.dma_start(io["dot1"], ot_sb[1])

    es.close()


def _rope_tables():
    inv_freq = 1.0 / (10000.0 ** (np.arange(0, D, 2, dtype=np.float64) / D))
    freqs = np.outer(np.arange(N, dtype=np.float64), inv_freq)  # [N, 32]
    cos_h = np.cos(freqs).astype(np.float32).T                  # [32, N]
    sin_h = np.sin(freqs).astype(np.float32).T                  # [32, N]
    cos64 = np.concatenate([cos_h, cos_h], 0)                   # [64, N]
    # swapped sin layout: row p holds the sin factor for the mul whose
    # *source* q rows sit at partition p (so both SBUF inputs share a base)
    sin64 = np.concatenate([sin_h, -sin_h], 0)                  # [64, N]
    cos2 = np.concatenate([cos64, cos64], 0)                    # [128, N]
    sin2 = np.concatenate([sin64, sin64], 0)                    # [128, N]
    return np.ascontiguousarray(cos2), np.ascontiguousarray(sin2)


@functools.lru_cache(maxsize=1)
def _program():
    nc = bacc.Bacc(
        "TRN2", target_bir_lowering=False, debug=False, enable_asserts=False
    )
    io = {
        "xt": nc.dram_tensor("xt", [NDB, 128, N], BF16, kind="ExternalInput").ap(),
        "wq": nc.dram_tensor("wq", [NDB, 128, 256], BF16, kind="ExternalInput").ap(),
        "wkv": nc.dram_tensor("wkv", [NDB, 128, 128], BF16, kind="ExternalInput").ap(),
        "wo": nc.dram_tensor("wo", [2, 128, DIM], BF16, kind="ExternalInput").ap(),
        "cos2": nc.dram_tensor("cos2", [128, N], BF16, kind="ExternalInput").ap(),
        "sin2": nc.dram_tensor("sin2", [128, N], BF16, kind="ExternalInput").ap(),
        "out": nc.dram_tensor("out", [N, DIM], F32, kind="ExternalOutput").ap(),
    }
    if DUMP:
        io["dqt"] = nc.dram_tensor("dqt", [128, QTOT], BF16, kind="ExternalOutput").ap()
        io["dkt"] = nc.dram_tensor("dkt", [128, N], BF16, kind="ExternalOutput").ap()
        io["dvaug"] = nc.dram_tensor("dvaug", [128, NKB * 128], BF16, kind="ExternalOutput").ap()
        io["dot0"] = nc.dram_tensor("dot0", [128, N], BF16, kind="ExternalOutput").ap()
        io["dot1"] = nc.dram_tensor("dot1", [128, N], BF16, kind="ExternalOutput").ap()
    with tile.TileContext(nc) as tc:
        build_kernel(nc, tc, io)
    nc.compile()
    return nc


def make_in_maps(x, Wq, Wkv, Wo):
    import ml_dtypes

    bf16 = ml_dtypes.bfloat16
    cos2, sin2 = _rope_tables()
    in_maps = []
    for c in range(8):
        b, j = c // 4, c % 4
        xt = np.ascontiguousarray(x[b].T).reshape(NDB, 128, N)
        wq_c = np.ascontiguousarray(Wq[:, 256 * j:256 * (j + 1)]).reshape(
            NDB, 128, 256
        )
        wkv_c = np.ascontiguousarray(
            np.concatenate(
                [Wkv[:, 64 * j:64 * (j + 1)],
                 Wkv[:, 256 + 64 * j:256 + 64 * (j + 1)]],
                axis=1,
            )
        ).reshape(NDB, 128, 128)
        wo_c = np.ascontiguousarray(Wo[256 * j:256 * (j + 1), :]).reshape(
            2, 128, DIM
        )
        in_maps.append(
            {
                "xt": xt.astype(bf16),
                "wq": wq_c.astype(bf16),
                "wkv": wkv_c.astype(bf16),
                "wo": wo_c.astype(bf16),
                "cos2": cos2.astype(bf16),
                "sin2": sin2.astype(bf16),
            }
        )
    return in_maps


def _install_ntff_hook():
    """Register the axon NTFF profiling hook that this image's antenv lacks."""
    import types

    if "antenv.axon_hooks" in sys.modules:
        return
    try:
        sys.path.append("/root/.axon_site")
        from trn_agent_boot.trn_boot import _ntff_profile_via_ctypes

        hook = _ntff_profile_via_ctypes("/opt/axon/libaxon_pjrt.so")
    except Exception:
        hook = None
    finally:
        try:
            sys.path.remove("/root/.axon_site")
        except ValueError:
            pass
    mod = types.ModuleType("antenv.axon_hooks")
    mod.get_axon_ntff_profile_hook = lambda: hook
    mod.set_axon_ntff_profile_hook = lambda h: None
    sys.modules["antenv.axon_hooks"] = mod
    # artifact upload needs bucket credentials this container lacks
    import concourse.bass_utils as bu

    bu.upload_artifacts = lambda tmpdir: "local://" + str(tmpdir)


def kernel(x, Wq, Wkv, Wo, bo):
    from concourse.bass_utils import run_bass_kernel_spmd

    _install_ntff_hook()
    nc = _program()
    in_maps = make_in_maps(x, Wq, Wkv, Wo)
    trace = bool(os.environ.get("KERNEL_TRACE"))
    res = run_bass_kernel_spmd(
        nc, in_maps, list(range(8)), trace=trace
    )
    LAST_RESULTS["res"] = res
    full = np.zeros((B, N, DIM), np.float32)
    for c in range(8):
        full[c // 4] += res.results[c]["out"]
    full += bo.astype(np.float32)
    return full
